# revision 16
# baseline (speedup 1.0000x reference)
"""EvaAttention TRN2 kernel: data-parallel over batch across 8 NeuronCores.

Per core (2 batches): qkv proj (fp32r matmuls), per-head QK layernorm + RoPE
(folded into host-precomputed cos/sin tables incl. scale and qn_g), attention
with no-max-subtraction softmax computed entirely in S^T layout (softmax
denominator via ones-augmented V column), scale_norm + proj.
"""
import os
import sys

for _p in (
    "/root/.axon_site",
    "/root/.axon_site/_ro/trn_rl_repo",
    "/root/.axon_site/_ro/pypackages",
    "/opt/trn_rl_repo",
    "/opt/pypackages",
):
    if os.path.isdir(_p) and _p not in sys.path:
        sys.path.append(_p)

import numpy as np

import concourse.bass as bass
import concourse.bacc as bacc
import concourse.tile as tile
from concourse import mybir, masks
from concourse.bass_utils import run_bass_kernel_spmd

F32 = mybir.dt.float32
F32R = mybir.dt.float32r
Act = mybir.ActivationFunctionType
Alu = mybir.AluOpType
X = mybir.AxisListType.X

B, N, C, H, D = 16, 1025, 1024, 16, 64
EPS = 1e-6
SCALE = D ** -0.5
NCORES = 8
BL = B // NCORES          # batches per core
NT = 9                    # token tiles per batch (pad 1025 -> 1152)
NPAD = NT * 128
HH = 2                    # head halves
HPH = H // HH             # heads per half (8)
PAIRS = HPH // 2          # head pairs per half (4)

_CACHE = {}


def _bcast_mid(ap2d, n):
    """[P, F] AP -> [P, n, F] with step-0 middle dim (free-dim broadcast)."""
    return bass.AP(tensor=ap2d.tensor, offset=ap2d.offset,
                   ap=[ap2d.ap[0], [0, n], ap2d.ap[1]])


def _build(has_kbias, has_pbias, repeat=1):
    nc = bacc.Bacc("TRN2", target_bir_lowering=False, debug=False,
                   num_devices=NCORES)

    x_in = nc.dram_tensor("x", [BL, N, C], F32R, kind="ExternalInput").ap()
    wt = nc.dram_tensor("wt", [C, 3 * C], F32R, kind="ExternalInput").ap()
    qkvb = nc.dram_tensor("qkvb", [3 * C], F32R, kind="ExternalInput").ap()
    ropet = nc.dram_tensor("ropet", [4, NPAD, D], F32R, kind="ExternalInput").ap()
    pwt = nc.dram_tensor("pwt", [C, C], F32R, kind="ExternalInput").ap()
    pbias = nc.dram_tensor("pbias", [C], F32R, kind="ExternalInput").ap()
    ident_d = nc.dram_tensor("ident", [128, 128], F32R,
                             kind="ExternalInput").ap()
    onesd = nc.dram_tensor("onesd", [1], F32R, kind="ExternalInput").ap()
    y = nc.dram_tensor("y", [BL, N, C], F32, kind="ExternalOutput").ap()

    with tile.TileContext(nc, pool_alloc_mode="queue") as tc:
        with tc.tile_pool(name="consts", bufs=1) as consts:
            ident = consts.tile([128, 128], F32R)
            nc.sync.dma_start(out=ident, in_=ident_d)
            epst = consts.tile([128, 1], F32)
            nc.vector.memset(epst, EPS)
            # rope tables: [0]=cos_q [1]=sin_q [2]=cos_k [3]=sin_k
            rtab = consts.tile([128, 4, NT, D], F32R)
            nc.sync.dma_start(
                out=rtab, in_=ropet.rearrange("f (t p) d -> p f t d", p=128))
            biasb = consts.tile([128, 3 * C], F32R)
            nc.sync.dma_start(
                out=biasb,
                in_=bass.AP(tensor=qkvb.tensor, offset=qkvb.offset,
                            ap=[[0, 128], [1, 3 * C]]))
            if has_pbias:
                pbb = consts.tile([128, C], F32R)
                nc.sync.dma_start(
                    out=pbb,
                    in_=bass.AP(tensor=pbias.tensor, offset=pbias.offset,
                                ap=[[0, 128], [1, C]]))

            nc._epst = epst
            nc._onesd = onesd
            import contextlib
            _psctx = contextlib.ExitStack()
            # [128,1024] 2-bank psum tiles for qkv t-pairs, S s-pairs, proj
            nc._mmps = _psctx.enter_context(
                tc.tile_pool(name="mmps", bufs=2, space="PSUM"))
            nc._pvps = _psctx.enter_context(
                tc.tile_pool(name="pvpsg", bufs=1, space="PSUM"))
            nc._trp = _psctx.enter_context(
                tc.tile_pool(name="trpg", bufs=2, space="PSUM"))
            for _rep in range(repeat):
                for b in range(BL):
                    _batch(nc, tc, b, x_in, wt, pwt, y, ident, rtab,
                           biasb, pbb if has_pbias else None, has_kbias)
            _psctx.close()
    nc.compile()
    return nc


def _batch(nc, tc, b, x_in, wt, pwt, y, ident, rtab, biasb, pbb,
           has_kbias):
    with tc.tile_pool(name="attnsb", bufs=1) as asp:
        # attention output, SBUF-resident: [token%128, t-tile, channel]
        attn_sb = asp.tile([128, NT, C], mybir.dt.bfloat16)
        # pad-tile rows (tokens 1026..1151) are never written; zero them so
        # the norm stats for t=8 read finite values
        nc.gpsimd.memset(attn_sb[:, NT - 1, :], 0.0)
        with tc.tile_pool(name="xt", bufs=1) as xtp:
            xT = [xtp.tile([128, NPAD], F32R, tag=f"xt{k}", name=f"xt{k}")
                  for k in range(8)]
            _build_xt(nc, tc, b, x_in, ident, xT)
            for hh in range(HH):
                with tc.tile_pool(name="qkt", bufs=1) as qktp:
                    QT = qktp.tile([128, PAIRS, NPAD], F32R)
                    KT = qktp.tile([128, PAIRS, NPAD], F32R)
                    V = qktp.tile([128, NT, HPH, D + 1], F32R)
                    _qkv_half(nc, tc, b, hh, wt, xT, ident, rtab, biasb, V,
                              QT, KT, has_kbias)
                    _attn_half(nc, tc, b, hh, QT, KT, V, ident, attn_sb)
        _norm_proj(nc, tc, b, pwt, attn_sb, ident, y, pbb)


def _build_xt(nc, tc, b, x_in, ident, xT):
    """Load x[b] and PE-transpose into xT [128c, 8k, NPAD tok]."""
    with tc.tile_pool(name="xraw", bufs=2) as xrp:
        xps = nc._trp
        for t in range(NT):
            xraw = xrp.tile([128, C], F32R)
            rows = 128 if t < NT - 1 else N - 128 * (NT - 1)
            nc.sync.dma_start(out=xraw[:rows, :],
                              in_=x_in[b, t * 128:t * 128 + rows, :])
            for k in range(8):
                ps = xps.tile([128, 128], F32R, tag="tr", name="xtr")
                nc.tensor.transpose(ps, xraw[:, k * 128:(k + 1) * 128],
                                    ident[:])
                if k % 2 == 0:
                    nc.scalar.copy(out=xT[k][:, t * 128:(t + 1) * 128],
                                   in_=ps)
                else:
                    nc.vector.tensor_copy(
                        out=xT[k][:, t * 128:(t + 1) * 128], in_=ps)


def _qkv_half(nc, tc, b, hh, wt, xT, ident, rtab, biasb, V, QT, KT,
              has_kbias):
    """qkv matmuls for one head-half + LN + RoPE + transposes into QT/KT/V."""
    with (
        tc.tile_pool(name="wp", bufs=2) as wp,
        tc.tile_pool(name="prep", bufs=2) as prep,
        tc.tile_pool(name="stat", bufs=4) as stp,
    ):
        qps, trps = nc._mmps, nc._trp
        # ones column of V (col D); untouched pad rows are never read
        nc.sync.dma_start(
            out=V[:, :, :, D:D + 1].rearrange("p t h o -> p (t h) o"),
            in_=bass.AP(tensor=nc._onesd.tensor, offset=nc._onesd.offset,
                        ap=[[0, 128], [0, NT * HPH], [1, 1]]))
        for oc in range(3):  # 0=q cols, 1=k cols, 2=v cols
            col0 = hh * 1536 + oc * 512
            wchunk = wp.tile([128, 8, 512], F32R)
            nc.sync.dma_start(
                out=wchunk,
                in_=wt[:, col0:col0 + 512].rearrange("(k p) o -> p k o",
                                                     p=128))
            for tp in range(0, NT, 2):
                pair = [t for t in (tp, tp + 1) if t < NT]
                psp = qps.tile([128, 1024], F32, tag="mm1024", name="qkvps")
                for ti, t in enumerate(pair):
                    for k in range(8):
                        nc.tensor.matmul(psp[:, ti * 512:ti * 512 + 512],
                                         xT[k][:, t * 128:(t + 1) * 128],
                                         wchunk[:, k, :], start=(k == 0),
                                         stop=(k == 7))
                for ti, t in enumerate(pair):
                    _qkv_post(nc, tc, hh, oc, t, psp[:, ti * 512:ti * 512 + 512],
                              rtab, biasb, V, QT, KT, has_kbias, prep, stp,
                              ident)


def _qkv_post(nc, tc, hh, oc, t, ps, rtab, biasb, V, QT, KT, has_kbias,
              prep, stp, ident):
    trps = nc._trp
    col0 = hh * 1536 + oc * 512
    if oc == 2:
        # v: bias add straight into V tile
        nc.vector.tensor_tensor(
            out=V[:, t, :, 0:D],
            in0=ps.rearrange("p (h d) -> p h d", h=HPH),
            in1=biasb[:, col0:col0 + 512].rearrange(
                "p (h d) -> p h d", h=HPH),
            op=Alu.add)
        return
    raw = prep.tile([128, HPH, D], F32, tag="raw")
    if oc == 1 and not has_kbias:
        nc.scalar.copy(out=raw.rearrange("p h d -> p (h d)"), in_=ps)
    else:
        nc.vector.tensor_tensor(
            out=raw.rearrange("p h d -> p (h d)"), in0=ps,
            in1=biasb[:, col0:col0 + 512], op=Alu.add)
    # ---- stats: mu, rstd per head ----
    sums = stp.tile([128, HPH], F32, tag="sums")
    nc.vector.tensor_reduce(sums, raw, axis=X, op=Alu.add)
    sq = prep.tile([128, HPH * D], F32, tag="sq")
    rawf = raw.rearrange("p h d -> p (h d)")
    nc.gpsimd.tensor_tensor(out=sq, in0=rawf, in1=rawf, op=Alu.mult)
    s2 = stp.tile([128, HPH], F32, tag="s2")
    nc.vector.tensor_reduce(
        s2, sq.rearrange("p (h d) -> p h d", h=HPH), axis=X, op=Alu.add)
    mu = stp.tile([128, HPH], F32, tag="mu")
    nc.vector.tensor_scalar(mu, sums, 1.0 / D, None, op0=Alu.mult)
    var = stp.tile([128, HPH], F32, tag="var")
    # var = s2/D - mu^2  (computed as (s2*(1/D) - mu*mu))
    nc.vector.tensor_tensor(out=var, in0=mu, in1=mu, op=Alu.mult)
    nc.vector.tensor_scalar(s2, s2, 1.0 / D, None, op0=Alu.mult)
    nc.vector.tensor_tensor(out=var, in0=s2, in1=var, op=Alu.subtract)
    sd = stp.tile([128, HPH], F32, tag="sd")
    nc.scalar.activation(sd, var, Act.Sqrt, bias=nc._epst[:, 0:1])
    rstd = stp.tile([128, HPH], F32, tag="rstd")
    nc.vector.reciprocal(rstd, sd)
    # ---- LN apply (gpsimd) ----
    ln = prep.tile([128, HPH, D], F32, tag="ln")
    for h in range(HPH):
        nc.gpsimd.tensor_scalar(
            ln[:, h, :], raw[:, h, :], mu[:, h:h + 1],
            rstd[:, h:h + 1], op0=Alu.subtract, op1=Alu.mult)
    # ---- RoPE: out = ln*COS + swap(ln)*SIN ----
    ctab = rtab[:, 2 * oc, t, :]      # cos_q or cos_k
    stab = rtab[:, 2 * oc + 1, t, :]  # sin_q or sin_k
    ra = prep.tile([128, HPH, D], F32, tag="ra")
    nc.gpsimd.tensor_tensor(out=ra, in0=ln, in1=_bcast_mid(ctab, HPH),
                            op=Alu.mult)
    rb = prep.tile([128, HPH, D], F32, tag="rb")
    half = D // 2
    nc.vector.tensor_tensor(
        out=rb[:, :, 0:half], in0=ln[:, :, half:D],
        in1=_bcast_mid(stab[:, 0:half], HPH), op=Alu.mult)
    nc.vector.tensor_tensor(
        out=rb[:, :, half:D], in0=ln[:, :, 0:half],
        in1=_bcast_mid(stab[:, half:D], HPH), op=Alu.mult)
    rot = prep.tile([128, HPH, D], F32R, tag="rot")
    nc.vector.tensor_tensor(out=rot, in0=ra, in1=rb, op=Alu.add)
    # ---- transpose head pairs into QT/KT ----
    dst = QT if oc == 0 else KT
    for p in range(PAIRS):
        tr = trps.tile([128, 128], F32R, tag="tr", name="qktr")
        nc.tensor.transpose(
            tr, rot.rearrange("p h d -> p (h d)")[:, p * 128:(p + 1) * 128],
            ident[:])
        if p % 2 == 0:
            nc.scalar.copy(out=dst[:, p, t * 128:(t + 1) * 128], in_=tr)
        else:
            nc.vector.tensor_copy(
                out=dst[:, p, t * 128:(t + 1) * 128], in_=tr)


def _attn_half(nc, tc, b, hh, QT, KT, V, ident, attn_sb):
    """Attention for 8 heads of one half (processed as 4 head pairs).

    The two heads of a pair occupy partition ranges 0:64 / 64:128 of the
    QT/KT pair tiles, so their S^T matmuls land on distinct PE row-groups
    (tile_position auto (0,0)/(64,0)) and run concurrently when emitted
    back-to-back."""
    with (
        tc.tile_pool(name="pt", bufs=6) as ptp,
        tc.tile_pool(name="att", bufs=3) as attp,
    ):
        sps, pvps, trp2 = nc._mmps, nc._pvps, nc._trp
        s8ps = trp2
        identF = ident[0:D + 1, 0:D + 1].bitcast(F32)
        for pp in range(PAIRS):
            heads = (2 * pp, 2 * pp + 1)
            for qc in range(2):
                q0 = qc * 512
                pvs = [pvps.tile([D + 1, 512], F32, tag=f"pv{s}",
                                 name=f"pv{s}") for s in range(2)]
                for kt in range(9):
                    # both heads' S^T tiles side by side in one 2-bank tile;
                    # kt==8 is the single key 1024 (1 valid partition row)
                    rows = 128 if kt < 8 else 1
                    k0, k1 = (kt * 128, kt * 128 + 128) if kt < 8 else (1024, 1025)
                    sp2 = sps.tile([128, 1024], F32, tag="mm1024", name="sp")
                    for s in range(2):
                        r = 64 * s
                        nc.tensor.matmul(
                            sp2[0:rows, s * 512:s * 512 + 512],
                            KT[r:r + 64, pp, k0:k1],
                            QT[r:r + 64, pp, q0:q0 + 512])
                    pt2 = ptp.tile([128, 1024], F32R, tag="pt")
                    nc.scalar.activation(pt2[0:rows, :], sp2[0:rows, :],
                                         Act.Exp)
                    for s, hl in enumerate(heads):
                        nc.tensor.matmul(pvs[s], V[0:rows, kt, hl, :],
                                         pt2[0:rows, s * 512:s * 512 + 512],
                                         start=(kt == 0), stop=(kt == 8))
                for s, hl in enumerate(heads):
                    hg = hh * HPH + hl
                    pvsb = attp.tile([D + 1, 512], F32, tag="pvs")
                    nc.vector.tensor_copy(out=pvsb, in_=pvs[s])
                    for j in range(4):
                        trf = trp2.tile([128, 128], F32R, tag="tr",
                                        name="atr")
                        tr = trf[:, 0:D + 1].bitcast(F32)
                        nc.tensor.transpose(
                            tr, pvsb[:, j * 128:(j + 1) * 128],
                            identF)
                        rl = attp.tile([128, 1], F32, tag="rl")
                        nc.vector.reciprocal(rl, tr[:, D:D + 1])
                        tt = qc * 4 + j
                        nc.vector.tensor_scalar(
                            attn_sb[:, tt, hg * D:(hg + 1) * D],
                            tr[:, 0:D], rl[:, 0:1], None, op0=Alu.mult)
            # ---- stragglers: q tokens 1023:1025 (token 1023 redone) ----
            sp1f = s8ps.tile([128, 128], F32R, tag="tr", name="sp1")
            sp1 = sp1f.bitcast(F32)[:, 0:36]
            for s in range(2):
                r = 64 * s
                qstr = QT[r:r + 64, pp, 1023:1025]
                for kt in range(8):
                    nc.tensor.matmul(
                        sp1[:, 18 * s + 2 * kt:18 * s + 2 * kt + 2],
                        KT[r:r + 64, pp, kt * 128:(kt + 1) * 128], qstr)
                nc.tensor.matmul(sp1[0:1, 18 * s + 16:18 * s + 18],
                                 KT[r:r + 64, pp, 1024:1025], qstr)
            p1w = ptp.tile([128, 36], F32R, tag="p1")
            nc.scalar.activation(p1w, sp1[:], Act.Exp)
            for s, hl in enumerate(heads):
                r = 64 * s
                hg = hh * HPH + hl
                p1 = p1w[:, 18 * s:18 * s + 18]
                pv1 = pvps.tile([D + 1, 2], F32, tag=f"pv{s}",
                                name=f"pv1_{s}")
                for kt in range(8):
                    nc.tensor.matmul(pv1, V[:, kt, hl, :],
                                     p1[:, 2 * kt:2 * kt + 2],
                                     start=(kt == 0), stop=False)
                nc.tensor.matmul(pv1, V[0:1, 8, hl, :], p1[0:1, 16:18],
                                 start=False, stop=True)
                pvs1 = attp.tile([D + 1, 2], F32, tag="pvs")
                nc.vector.tensor_copy(out=pvs1, in_=pv1)
                # only token 1024 is new (1023 was redone for the matmul's
                # sake but is already written); transposing col 1 alone puts
                # it at partition 0 = its slot in t-tile 8
                trf1 = trp2.tile([128, 128], F32R, tag="tr", name="atr1")
                tr1 = trf1[:, 0:D + 1].bitcast(F32)
                nc.tensor.transpose(tr1[0:1, :], pvs1[:, 1:2], identF)
                rl1 = attp.tile([128, 1], F32, tag="rl")
                nc.vector.reciprocal(rl1[0:1, :], tr1[0:1, D:D + 1])
                nc.vector.tensor_scalar(
                    attn_sb[0:1, 8, hg * D:(hg + 1) * D], tr1[0:1, 0:D],
                    rl1[0:1, 0:1], None, op0=Alu.mult)


def _norm_proj(nc, tc, b, pwt, attn_sb, ident, y, pbb):
    """scale_norm over C + proj matmul + output DMA for batch b."""
    with (
        tc.tile_pool(name="lnt", bufs=1) as lntp,
        tc.tile_pool(name="ain", bufs=3) as ainp,
        tc.tile_pool(name="lst", bufs=6) as lstp,
    ):
        lps = nc._mmps
        lnT = lntp.tile([128, 8, NPAD], F32R)
        for t in range(NT):
            a = attn_sb[:, t, :]
            s = lstp.tile([128, 1], F32, tag="s")
            nc.vector.tensor_reduce(s, a, axis=X, op=Alu.add)
            sq = ainp.tile([128, C], F32, tag="lsq")
            nc.gpsimd.tensor_tensor(out=sq, in0=a, in1=a, op=Alu.mult)
            s2 = lstp.tile([128, 1], F32, tag="ls2")
            nc.vector.tensor_reduce(s2, sq, axis=X, op=Alu.add)
            mu = lstp.tile([128, 1], F32, tag="lmu")
            nc.vector.tensor_scalar(mu, s, 1.0 / C, None, op0=Alu.mult)
            var = lstp.tile([128, 1], F32, tag="lvar")
            nc.vector.tensor_tensor(out=var, in0=mu, in1=mu, op=Alu.mult)
            nc.vector.tensor_scalar(s2, s2, 1.0 / C, None, op0=Alu.mult)
            nc.vector.tensor_tensor(out=var, in0=s2, in1=var,
                                    op=Alu.subtract)
            sd = lstp.tile([128, 1], F32, tag="lsd")
            nc.scalar.activation(sd, var, Act.Sqrt, bias=nc._epst[:, 0:1])
            rstd = lstp.tile([128, 1], F32, tag="lrstd")
            nc.vector.reciprocal(rstd, sd)
            ln = ainp.tile([128, C], F32R, tag="ln2")
            nc.vector.tensor_scalar(ln, a, mu[:, 0:1], rstd[:, 0:1],
                                    op0=Alu.subtract, op1=Alu.mult)
            if True:
                ltps = nc._trp
                for k in range(8):
                    tp = ltps.tile([128, 128], F32R, tag="tr", name="lntr")
                    nc.tensor.transpose(tp, ln[:, k * 128:(k + 1) * 128],
                                        ident[:])
                    if k % 2 == 0:
                        nc.scalar.copy(out=lnT[:, k, t * 128:(t + 1) * 128],
                                       in_=tp)
                    else:
                        nc.vector.tensor_copy(
                            out=lnT[:, k, t * 128:(t + 1) * 128], in_=tp)
        with tc.tile_pool(name="pwp", bufs=2) as pwp:
            for oc in range(2):
                wchunk = pwp.tile([128, 8, 512], F32R)
                nc.sync.dma_start(
                    out=wchunk,
                    in_=pwt[:, oc * 512:(oc + 1) * 512].rearrange(
                        "(k p) o -> p k o", p=128))
                for tp in range(0, NT, 2):
                    pair = [t for t in (tp, tp + 1) if t < NT]
                    psp = lps.tile([128, 1024], F32, tag="mm1024",
                                   name="projps")
                    for ti, t in enumerate(pair):
                        for k in range(8):
                            nc.tensor.matmul(
                                psp[:, ti * 512:ti * 512 + 512],
                                lnT[:, k, t * 128:(t + 1) * 128],
                                wchunk[:, k, :], start=(k == 0),
                                stop=(k == 7))
                    for ti, t in enumerate(pair):
                        ps = psp[:, ti * 512:ti * 512 + 512]
                        ostage = ainp.tile([128, 512], F32, tag="ostage")
                        if pbb is not None:
                            nc.vector.tensor_tensor(
                                out=ostage, in0=ps,
                                in1=pbb[:, oc * 512:(oc + 1) * 512],
                                op=Alu.add)
                        else:
                            nc.scalar.copy(out=ostage, in_=ps)
                        rows = 128 if t < NT - 1 else N - 128 * (NT - 1)
                        nc.sync.dma_start(
                            out=y[b, t * 128:t * 128 + rows,
                                  oc * 512:(oc + 1) * 512],
                            in_=ostage[:rows, :])


def _host_prep(inputs):
    """Precompute permuted/transposed weights and folded rope tables."""
    perm = np.concatenate([np.arange(0, D, 2), np.arange(1, D, 2)])
    swap = np.concatenate([np.arange(D // 2, D), np.arange(0, D // 2)])

    qkv_w = np.asarray(inputs["qkv_w"], np.float32)
    rope = np.asarray(inputs["rope"], np.float32)
    sin_t, cos_t = rope[:, :D], rope[:, D:]

    # column order: [half][q|k|v][head-in-half][d]  (d permuted for q,k)
    row_order = np.empty(3 * C, np.int64)
    col = 0
    for hh in range(HH):
        for grp in range(3):
            for h in range(hh * HPH, (hh + 1) * HPH):
                base = grp * C + h * D
                idx = base + (perm if grp < 2 else np.arange(D))
                row_order[col:col + D] = idx
                col += D
    wt = np.ascontiguousarray(qkv_w[row_order, :].T)  # [C, 3C]

    qb = np.asarray(inputs["q_bias"], np.float32)
    kb = np.asarray(inputs["k_bias"], np.float32)
    vb = np.asarray(inputs["v_bias"], np.float32)
    full_bias = np.concatenate([qb, kb, vb])
    qkvb = full_bias[row_order].astype(np.float32)

    def make_tables(g, scale):
        gp = np.asarray(g, np.float32)[perm]          # g in permuted coords
        cos_p = cos_t[:, perm]                        # [1024, D]
        sin_p = sin_t[:, perm]
        sgn = np.where(np.arange(D) < D // 2, -1.0, 1.0).astype(np.float32)
        cost = np.zeros((NPAD, D), np.float32)
        sint = np.zeros((NPAD, D), np.float32)
        cost[0] = gp * scale
        cost[1:N] = cos_p * gp[None, :] * scale
        sint[1:N] = sin_p * sgn[None, :] * gp[swap][None, :] * scale
        return cost, sint

    cq, sq_ = make_tables(inputs["qn_g"], SCALE)
    ck, sk = make_tables(inputs["kn_g"], 1.0)
    ropet = np.stack([cq, sq_, ck, sk])  # [4, NPAD, D]

    norm_g = np.asarray(inputs["norm_g"], np.float32)
    norm_b = np.asarray(inputs["norm_b"], np.float32)
    proj_w = np.asarray(inputs["proj_w"], np.float32)
    proj_b = np.asarray(inputs["proj_b"], np.float32)
    pwt = np.ascontiguousarray((proj_w * norm_g[None, :]).T)  # [C, C]
    pbias = (proj_b + norm_b @ proj_w.T).astype(np.float32)

    return wt, qkvb, ropet, pwt, pbias


def kernel(**inputs):
    qn_b = np.asarray(inputs["qn_b"], np.float32)
    kn_b = np.asarray(inputs["kn_b"], np.float32)
    assert not qn_b.any() and not kn_b.any(), \
        "kernel specialized for qn_b == kn_b == 0"

    wt, qkvb, ropet, pwt, pbias = _host_prep(inputs)
    has_kbias = bool(np.asarray(inputs["k_bias"]).any())
    has_pbias = bool(pbias.any())

    key = (has_kbias, has_pbias)
    if key not in _CACHE:
        _CACHE[key] = _build(has_kbias, has_pbias)
    nc = _CACHE[key]

    x = np.asarray(inputs["x"], np.float32)
    in_maps = []
    for c in range(NCORES):
        in_maps.append({
            "x": np.ascontiguousarray(x[c * BL:(c + 1) * BL]),
            "wt": wt, "qkvb": qkvb, "ropet": ropet, "pwt": pwt,
            "pbias": pbias, "ident": np.eye(128, dtype=np.float32),
            "onesd": np.ones(1, dtype=np.float32),
        })
    res = run_bass_kernel_spmd(nc, in_maps, core_ids=list(range(NCORES)))
    out = np.concatenate([res.results[c]["y"] for c in range(NCORES)], axis=0)
    return out.astype(np.float32)



# revision 19
# speedup vs baseline: 1.0196x; 1.0196x over previous
"""EvaAttention TRN2 kernel: data-parallel over batch across 8 NeuronCores.

Per core (2 batches): qkv proj, per-head QK layernorm + RoPE (folded into
host-precomputed cos/sin tables incl. scale and qn_g), attention with
no-max-subtraction softmax in S^T layout (denominator via ones-augmented V
column), scale_norm + proj.  bf16 operand storage throughout (fp32 PSUM
accumulation), attention output kept SBUF-resident, and the per-half phases
are emitted interleaved (attention of half h overlaps qkv/LN/RoPE of half
h+1, next batch's x-transposes, and the previous batch's norm+proj) so no
engine sits idle behind another phase's bottleneck.
"""
import os
import sys

for _p in (
    "/root/.axon_site",
    "/root/.axon_site/_ro/trn_rl_repo",
    "/root/.axon_site/_ro/pypackages",
    "/opt/trn_rl_repo",
    "/opt/pypackages",
):
    if os.path.isdir(_p) and _p not in sys.path:
        sys.path.append(_p)

import numpy as np

import concourse.bass as bass
import concourse.bacc as bacc
import concourse.tile as tile
from concourse import mybir
from concourse.bass_utils import run_bass_kernel_spmd

F32 = mybir.dt.float32
F32R = mybir.dt.float32r
BF16 = mybir.dt.bfloat16
FP16 = mybir.dt.float16
Act = mybir.ActivationFunctionType
Alu = mybir.AluOpType
X = mybir.AxisListType.X

B, N, C, H, D = 16, 1025, 1024, 16, 64
EPS = 1e-6
SCALE = D ** -0.5
NCORES = 8
BL = B // NCORES          # batches per core
NT = 9                    # token tiles per batch (pad 1025 -> 1152)
NPAD = NT * 128
HH = 2                    # head halves
HPH = H // HH             # heads per half (8)
PAIRS = HPH // 2          # head pairs per half (4)

_CACHE = {}


def _bcast_mid(ap2d, n):
    """[P, F] AP -> [P, n, F] with step-0 middle dim (free-dim broadcast)."""
    return bass.AP(tensor=ap2d.tensor, offset=ap2d.offset,
                   ap=[ap2d.ap[0], [0, n], ap2d.ap[1]])


def _ilv(gens):
    """Fair-merge emission: step the generator with least fractional
    progress; n-units given per generator."""
    state = []
    for g, n in gens:
        state.append([g, max(n, 1), 0, False])
    while True:
        live = [s for s in state if not s[3]]
        if not live:
            return
        s = min(live, key=lambda s: s[2] / s[1])
        try:
            next(s[0])
            s[2] += 1
        except StopIteration:
            s[3] = True


class K:
    def __init__(self, nc, tc, stk, has_kbias, has_pbias):
        self.nc, self.tc = nc, tc
        self.has_kbias, self.has_pbias = has_kbias, has_pbias
        p = tc.tile_pool
        e = stk.enter_context
        # PSUM: qmm 2x[128,1024] (qkv/proj matmuls + transpose scratch),
        # smm 1x (S tiles + attn-out transpose scratch), pv 1x = 8 banks
        self.qps = e(p(name="qps", bufs=1, space="PSUM"))
        self.sps = e(p(name="sps", bufs=1, space="PSUM"))
        self.pvp = e(p(name="pvp", bufs=1, space="PSUM"))
        self.trp = e(p(name="trp", bufs=2, space="PSUM"))
        # SBUF pools
        self.xtp = e(p(name="xtp", bufs=1))
        self.qkp = e(p(name="qkp", bufs=2))
        self.asp = e(p(name="asp", bufs=1))
        self.lnp = e(p(name="lnp", bufs=1))
        self.wch = e(p(name="wch", bufs=3))
        self.prep = e(p(name="prep", bufs=3))
        self.stp = e(p(name="stp", bufs=4))
        self.ptp = e(p(name="ptp", bufs=3))
        self.attp = e(p(name="attp", bufs=3))
        self.ainp = e(p(name="ainp", bufs=3))
        self.lstp = e(p(name="lstp", bufs=6))
        self.xrp = e(p(name="xrp", bufs=2))

    def alloc_qkv(self):
        QT = self.qkp.tile([128, PAIRS, NPAD], FP16, tag="QT", name="QT")
        KT = self.qkp.tile([128, PAIRS, NPAD], FP16, tag="KT", name="KT")
        V = self.qkp.tile([128, NT, HPH, D + 1], BF16, tag="V", name="V")
        return QT, KT, V

    def alloc_xt(self):
        return [self.xtp.tile([128, NPAD], FP16, tag=f"xt{k}", name=f"xt{k}")
                for k in range(8)]

    def alloc_asb(self):
        return self.asp.tile([128, NT, C], FP16, tag="asb", name="asb")

    # ---------------- x load + transpose ----------------
    def gen_x(self, b, xT):
        nc = self.nc
        for t in range(NT):
            xraw = self.xrp.tile([128, C], F32R, tag="xraw")
            rows = 128 if t < NT - 1 else N - 128 * (NT - 1)
            nc.sync.dma_start(out=xraw[:rows, :],
                              in_=self.x_in[b, t * 128:t * 128 + rows, :])
            xcvt = self.xrp.tile([128, C], FP16, tag="xcvt")
            nc.gpsimd.tensor_copy(out=xcvt, in_=xraw)
            trb = self.trp.tile([128, 1024], FP16, tag="trb", name="xtrb")
            for k in range(8):
                ps = trb[:, k * 128:(k + 1) * 128]
                nc.tensor.transpose(ps, xcvt[:, k * 128:(k + 1) * 128],
                                    self.ident16[:])
                if k % 2 == 0:
                    nc.scalar.copy(out=xT[k][:, t * 128:(t + 1) * 128],
                                   in_=ps)
                else:
                    nc.vector.tensor_copy(
                        out=xT[k][:, t * 128:(t + 1) * 128], in_=ps)
            yield

    # ---------------- qkv + LN + RoPE ----------------
    def gen_qkv(self, b, hh, xT, QT, KT, V):
        nc = self.nc
        nc.sync.dma_start(
            out=V[:, :, :, D:D + 1].rearrange("p t h o -> p (t h) o"),
            in_=bass.AP(tensor=self.onesd.tensor, offset=self.onesd.offset,
                        ap=[[0, 128], [0, NT * HPH], [1, 1]]))
        for oc in range(3):  # 0=q, 1=k, 2=v
            col0 = hh * 1536 + oc * 512
            wc = self.wch.tile([128, 8, 512], FP16, tag="wch", name="wch")
            nc.sync.dma_start(
                out=wc,
                in_=self.wt[:, col0:col0 + 512].rearrange("(k p) o -> p k o",
                                                          p=128))
            for tp in range(0, NT, 2):
                pair = [t for t in (tp, tp + 1) if t < NT]
                psp = self.qps.tile([128, 1024], F32, tag="qmm", name="qkvps")
                for ti, t in enumerate(pair):
                    for k in range(8):
                        nc.tensor.matmul(psp[:, ti * 512:ti * 512 + 512],
                                         xT[k][:, t * 128:(t + 1) * 128],
                                         wc[:, k, :], start=(k == 0),
                                         stop=(k == 7))
                if oc < 2:
                    trb = self.trp.tile([128, 1024], FP16, tag="trb",
                                        name="qktrb")
                for ti, t in enumerate(pair):
                    self._qkv_post(hh, oc, t, psp[:, ti * 512:ti * 512 + 512],
                                   trb if oc < 2 else None,
                                   ti, QT, KT, V)
                yield

    def _qkv_post(self, hh, oc, t, ps, trb16, ti, QT, KT, V):
        nc = self.nc
        col0 = hh * 1536 + oc * 512
        if oc == 2:
            nc.vector.tensor_tensor(
                out=V[:, t, :, 0:D],
                in0=ps.rearrange("p (h d) -> p h d", h=HPH),
                in1=self.biasb[:, col0:col0 + 512].rearrange(
                    "p (h d) -> p h d", h=HPH),
                op=Alu.add)
            return
        raw = self.prep.tile([128, HPH, D], FP16, tag="raw")
        if oc == 1 and not self.has_kbias:
            nc.scalar.copy(out=raw.rearrange("p h d -> p (h d)"), in_=ps)
        else:
            nc.vector.tensor_tensor(
                out=raw.rearrange("p h d -> p (h d)"), in0=ps,
                in1=self.biasb[:, col0:col0 + 512], op=Alu.add)
        # stats over head_dim
        sums = self.stp.tile([128, HPH], F32, tag="sums")
        nc.vector.tensor_reduce(sums, raw, axis=X, op=Alu.add)
        sq = self.prep.tile([128, HPH * D], FP16, tag="sq")
        rawf = raw.rearrange("p h d -> p (h d)")
        nc.vector.tensor_tensor(out=sq, in0=rawf, in1=rawf, op=Alu.mult)
        s2 = self.stp.tile([128, HPH], F32, tag="s2")
        nc.vector.tensor_reduce(
            s2, sq.rearrange("p (h d) -> p h d", h=HPH), axis=X, op=Alu.add)
        mu = self.stp.tile([128, HPH], F32, tag="mu")
        nc.vector.tensor_scalar(mu, sums, 1.0 / D, None, op0=Alu.mult)
        var = self.stp.tile([128, HPH], F32, tag="var")
        nc.vector.tensor_tensor(out=var, in0=mu, in1=mu, op=Alu.mult)
        nc.vector.tensor_scalar(s2, s2, 1.0 / D, None, op0=Alu.mult)
        nc.vector.tensor_tensor(out=var, in0=s2, in1=var, op=Alu.subtract)
        sd = self.stp.tile([128, HPH], F32, tag="sd")
        nc.scalar.activation(sd, var, Act.Sqrt, bias=self.epst[:, 0:1])
        rstd = self.stp.tile([128, HPH], F32, tag="rstd")
        nc.vector.reciprocal(rstd, sd)
        # LN apply: 4 heads on gpsimd, 4 on DVE (bf16 fast path)
        ln = self.prep.tile([128, HPH, D], FP16, tag="ln")
        for h in range(HPH):
            eng = nc.gpsimd if h % 2 == 0 else nc.vector
            eng.tensor_scalar(
                ln[:, h, :], raw[:, h, :], mu[:, h:h + 1],
                rstd[:, h:h + 1], op0=Alu.subtract, op1=Alu.mult)
        # RoPE
        ctab = self.rtab[:, 2 * oc, t, :]
        stab = self.rtab[:, 2 * oc + 1, t, :]
        ra = self.prep.tile([128, HPH, D], FP16, tag="ra")
        nc.gpsimd.tensor_tensor(out=ra, in0=ln, in1=_bcast_mid(ctab, HPH),
                                op=Alu.mult)
        rb = self.prep.tile([128, HPH, D], FP16, tag="rb")
        half = D // 2
        nc.vector.tensor_tensor(
            out=rb[:, :, 0:half], in0=ln[:, :, half:D],
            in1=_bcast_mid(stab[:, 0:half], HPH), op=Alu.mult)
        nc.vector.tensor_tensor(
            out=rb[:, :, half:D], in0=ln[:, :, 0:half],
            in1=_bcast_mid(stab[:, half:D], HPH), op=Alu.mult)
        rot = self.prep.tile([128, HPH, D], FP16, tag="rot")
        nc.vector.tensor_tensor(out=rot, in0=ra, in1=rb, op=Alu.add)
        # transpose head pairs into QT/KT (bf16: 8 slots per trb tile)
        dst = QT if oc == 0 else KT
        for p in range(PAIRS):
            sl = ti * 4 + p
            tr = trb16[:, sl * 128:(sl + 1) * 128]
            nc.tensor.transpose(
                tr, rot.rearrange("p h d -> p (h d)")[:, p * 128:(p + 1) * 128],
                self.ident16[:])
            if p % 2 == 0:
                nc.scalar.copy(out=dst[:, p, t * 128:(t + 1) * 128], in_=tr)
            else:
                nc.vector.tensor_copy(
                    out=dst[:, p, t * 128:(t + 1) * 128], in_=tr)

    # ---------------- attention ----------------
    def gen_attn(self, b, hh, QT, KT, V, attn_sb):
        nc = self.nc
        identF = self.ident[0:D + 1, 0:D + 1].bitcast(F32)
        for pp in range(PAIRS):
            heads = (2 * pp, 2 * pp + 1)
            for qc in range(2):
                q0 = qc * 512
                pv = self.pvp.tile([D + 1, 1024], F32, tag="pv", name="pv")
                for kt in range(9):
                    rows = 128 if kt < 8 else 1
                    k0, k1 = ((kt * 128, kt * 128 + 128) if kt < 8
                              else (1024, 1025))
                    sp2 = self.sps.tile([128, 1024], F32, tag="smm",
                                        name="sp")
                    for s in range(2):
                        r = 64 * s
                        nc.tensor.matmul(
                            sp2[0:rows, s * 512:s * 512 + 512],
                            KT[r:r + 64, pp, k0:k1],
                            QT[r:r + 64, pp, q0:q0 + 512])
                    pt2 = self.ptp.tile([128, 1024], BF16, tag="pt")
                    nc.scalar.activation(pt2[0:rows, :], sp2[0:rows, :],
                                         Act.Exp)
                    for s, hl in enumerate(heads):
                        nc.tensor.matmul(pv[:, s * 512:s * 512 + 512],
                                         V[0:rows, kt, hl, :],
                                         pt2[0:rows, s * 512:s * 512 + 512],
                                         start=(kt == 0), stop=(kt == 8))
                    yield
                for s, hl in enumerate(heads):
                    trbf = self.trp.tile([128, 1024], FP16, tag="trb",
                                         name="atrb").bitcast(F32)
                    hg = hh * HPH + hl
                    pvsb = self.attp.tile([D + 1, 512], F32, tag="pvs")
                    nc.vector.tensor_copy(out=pvsb,
                                          in_=pv[:, s * 512:s * 512 + 512])
                    for j in range(4):
                        c0 = j * 128
                        tr = trbf[:, c0:c0 + 65]
                        nc.tensor.transpose(
                            tr, pvsb[:, j * 128:(j + 1) * 128], identF)
                        rl = self.attp.tile([128, 1], F32, tag="rl")
                        nc.vector.reciprocal(rl, tr[:, D:D + 1])
                        tt = qc * 4 + j
                        nc.vector.tensor_scalar(
                            attn_sb[:, tt, hg * D:(hg + 1) * D],
                            tr[:, 0:D], rl[:, 0:1], None, op0=Alu.mult)
                    yield
            # stragglers: q tokens 1023:1025 (1023 redone, only 1024 kept)
            trbs = self.trp.tile([128, 1024], FP16, tag="trb",
                                 name="strb").bitcast(F32)
            sp1 = trbs[:, 0:36]
            for s in range(2):
                r = 64 * s
                qstr = QT[r:r + 64, pp, 1023:1025]
                for kt in range(8):
                    nc.tensor.matmul(
                        sp1[:, 18 * s + 2 * kt:18 * s + 2 * kt + 2],
                        KT[r:r + 64, pp, kt * 128:(kt + 1) * 128], qstr)
                nc.tensor.matmul(sp1[0:1, 18 * s + 16:18 * s + 18],
                                 KT[r:r + 64, pp, 1024:1025], qstr)
            p1w = self.ptp.tile([128, 36], BF16, tag="p1")
            nc.scalar.activation(p1w, sp1[:], Act.Exp)
            for s, hl in enumerate(heads):
                hg = hh * HPH + hl
                p1 = p1w[:, 18 * s:18 * s + 18]
                pv1 = self.pvp.tile([D + 1, 1024], F32, tag="pv",
                                    name="pv1")
                for kt in range(8):
                    nc.tensor.matmul(pv1[:, 0:2], V[:, kt, hl, :],
                                     p1[:, 2 * kt:2 * kt + 2],
                                     start=(kt == 0), stop=False)
                nc.tensor.matmul(pv1[:, 0:2], V[0:1, 8, hl, :],
                                 p1[0:1, 16:18], start=False, stop=True)
                pvs1 = self.attp.tile([D + 1, 2], F32, tag="pvs")
                nc.vector.tensor_copy(out=pvs1, in_=pv1[:, 0:2])
                tr1 = trbs[:, 256 + s * 128:256 + s * 128 + 65]
                nc.tensor.transpose(tr1[0:1, :], pvs1[:, 1:2], identF)
                rl1 = self.attp.tile([128, 1], F32, tag="rl")
                nc.vector.reciprocal(rl1[0:1, :], tr1[0:1, D:D + 1])
                nc.vector.tensor_scalar(
                    attn_sb[0:1, 8, hg * D:(hg + 1) * D], tr1[0:1, 0:D],
                    rl1[0:1, 0:1], None, op0=Alu.mult)
            yield

    # ---------------- scale_norm + proj ----------------
    def gen_norm(self, b, attn_sb):
        nc = self.nc
        lnT = self.lnp.tile([128, 8, NPAD], FP16, tag="lnT", name="lnT")
        for t in range(NT):
            a = attn_sb[:, t, :]
            s = self.lstp.tile([128, 1], F32, tag="s")
            nc.vector.tensor_reduce(s, a, axis=X, op=Alu.add)
            sq = self.ainp.tile([128, C], FP16, tag="lsq")
            nc.gpsimd.tensor_tensor(out=sq, in0=a, in1=a, op=Alu.mult)
            s2 = self.lstp.tile([128, 1], F32, tag="ls2")
            nc.vector.tensor_reduce(s2, sq, axis=X, op=Alu.add)
            mu = self.lstp.tile([128, 1], F32, tag="lmu")
            nc.vector.tensor_scalar(mu, s, 1.0 / C, None, op0=Alu.mult)
            var = self.lstp.tile([128, 1], F32, tag="lvar")
            nc.vector.tensor_tensor(out=var, in0=mu, in1=mu, op=Alu.mult)
            nc.vector.tensor_scalar(s2, s2, 1.0 / C, None, op0=Alu.mult)
            nc.vector.tensor_tensor(out=var, in0=s2, in1=var,
                                    op=Alu.subtract)
            sd = self.lstp.tile([128, 1], F32, tag="lsd")
            nc.scalar.activation(sd, var, Act.Sqrt, bias=self.epst[:, 0:1])
            rstd = self.lstp.tile([128, 1], F32, tag="lrstd")
            nc.vector.reciprocal(rstd, sd)
            ln = self.ainp.tile([128, C], FP16, tag="ln2")
            nc.vector.tensor_scalar(ln, a, mu[:, 0:1], rstd[:, 0:1],
                                    op0=Alu.subtract, op1=Alu.mult)
            trb16 = self.trp.tile([128, 1024], FP16, tag="trb",
                                  name="ltrb")
            for k in range(8):
                tr = trb16[:, k * 128:(k + 1) * 128]
                nc.tensor.transpose(tr, ln[:, k * 128:(k + 1) * 128],
                                    self.ident16[:])
                if k % 2 == 0:
                    nc.scalar.copy(out=lnT[:, k, t * 128:(t + 1) * 128],
                                   in_=tr)
                else:
                    nc.vector.tensor_copy(
                        out=lnT[:, k, t * 128:(t + 1) * 128], in_=tr)
            yield
        for oc in range(2):
            wc = self.wch.tile([128, 8, 512], FP16, tag="wch", name="pwch")
            nc.sync.dma_start(
                out=wc,
                in_=self.pwt[:, oc * 512:(oc + 1) * 512].rearrange(
                    "(k p) o -> p k o", p=128))
            for tp in range(0, NT, 2):
                pair = [t for t in (tp, tp + 1) if t < NT]
                psp = self.qps.tile([128, 1024], F32, tag="qmm",
                                    name="projps")
                for ti, t in enumerate(pair):
                    for k in range(8):
                        nc.tensor.matmul(
                            psp[:, ti * 512:ti * 512 + 512],
                            lnT[:, k, t * 128:(t + 1) * 128],
                            wc[:, k, :], start=(k == 0), stop=(k == 7))
                for ti, t in enumerate(pair):
                    ps = psp[:, ti * 512:ti * 512 + 512]
                    ostage = self.ainp.tile([128, 512], F32, tag="ostage")
                    if self.pbb is not None:
                        nc.vector.tensor_tensor(
                            out=ostage, in0=ps,
                            in1=self.pbb[:, oc * 512:(oc + 1) * 512],
                            op=Alu.add)
                    else:
                        nc.scalar.copy(out=ostage, in_=ps)
                    rows = 128 if t < NT - 1 else N - 128 * (NT - 1)
                    nc.sync.dma_start(
                        out=self.y[b, t * 128:t * 128 + rows,
                                   oc * 512:(oc + 1) * 512],
                        in_=ostage[:rows, :])
                yield


def _empty():
    return
    yield


def _build(has_kbias, has_pbias, repeat=1):
    nc = bacc.Bacc("TRN2", target_bir_lowering=False, debug=False,
                   num_devices=NCORES)

    x_in = nc.dram_tensor("x", [BL, N, C], F32R, kind="ExternalInput").ap()
    wt = nc.dram_tensor("wt", [C, 3 * C], FP16, kind="ExternalInput").ap()
    qkvb = nc.dram_tensor("qkvb", [3 * C], FP16, kind="ExternalInput").ap()
    ropet = nc.dram_tensor("ropet", [4, NPAD, D], FP16,
                           kind="ExternalInput").ap()
    pwt = nc.dram_tensor("pwt", [C, C], FP16, kind="ExternalInput").ap()
    pbias = nc.dram_tensor("pbias", [C], F32R, kind="ExternalInput").ap()
    ident_d = nc.dram_tensor("ident", [128, 128], F32R,
                             kind="ExternalInput").ap()
    onesd = nc.dram_tensor("onesd", [1], BF16, kind="ExternalInput").ap()
    y = nc.dram_tensor("y", [BL, N, C], F32, kind="ExternalOutput").ap()

    import contextlib
    with tile.TileContext(nc, pool_alloc_mode="queue") as tc:
        with contextlib.ExitStack() as stk:
            consts = stk.enter_context(tc.tile_pool(name="consts", bufs=1))
            ident = consts.tile([128, 128], F32R)
            nc.sync.dma_start(out=ident, in_=ident_d)
            ident16 = consts.tile([128, 128], FP16)
            nc.vector.tensor_copy(out=ident16, in_=ident)
            epst = consts.tile([128, 1], F32)
            nc.vector.memset(epst, EPS)
            rtab = consts.tile([128, 4, NT, D], FP16)
            nc.sync.dma_start(
                out=rtab, in_=ropet.rearrange("f (t p) d -> p f t d", p=128))
            biasb = consts.tile([128, 3 * C], FP16)
            nc.sync.dma_start(
                out=biasb,
                in_=bass.AP(tensor=qkvb.tensor, offset=qkvb.offset,
                            ap=[[0, 128], [1, 3 * C]]))
            pbb = None
            if has_pbias:
                pbb = consts.tile([128, C], F32R)
                nc.sync.dma_start(
                    out=pbb,
                    in_=bass.AP(tensor=pbias.tensor, offset=pbias.offset,
                                ap=[[0, 128], [1, C]]))

            k = K(nc, tc, stk, has_kbias, has_pbias)
            k.x_in, k.wt, k.pwt, k.y = x_in, wt, pwt, y
            k.onesd, k.ident, k.ident16 = onesd, ident, ident16
            k.epst, k.rtab, k.biasb, k.pbb = epst, rtab, biasb, pbb

            for _rep in range(repeat):
                # prologue: batch 0 x-transposes + first qkv half
                xT = k.alloc_xt()
                for _ in k.gen_x(0, xT):
                    pass
                QKV0 = k.alloc_qkv()
                for _ in k.gen_qkv(0, 0, xT, *QKV0):
                    pass
                gN_prev = None
                for b in range(BL):
                    asb = k.alloc_asb()
                    nc.gpsimd.memset(asb[:, NT - 1, :], 0.0)
                    # beta: attention(half0) || qkv(half1) [|| prev norm tail]
                    QKV1 = k.alloc_qkv()
                    gens = [(k.gen_attn(b, 0, *QKV0, asb), 76),
                            (k.gen_qkv(b, 1, xT, *QKV1), 15)]
                    if gN_prev is not None:
                        gens.append((gN_prev, 19))
                    _ilv(gens)
                    # gamma: attention(half1) || next batch x-transposes
                    if b + 1 < BL:
                        xT = k.alloc_xt()
                        gx = k.gen_x(b + 1, xT)
                    else:
                        gx = _empty()
                    _ilv([(k.gen_attn(b, 1, *QKV1, asb), 76), (gx, 9)])
                    # alpha: norm+proj(b) || qkv(b+1, half0)
                    gN = k.gen_norm(b, asb)
                    if b + 1 < BL:
                        QKV0 = k.alloc_qkv()
                        _ilv([(k.gen_qkv(b + 1, 0, xT, *QKV0), 15),
                              (gN, 19)])
                        gN_prev = gN  # exhausted or not; _ilv drains fully
                        gN_prev = None
                    else:
                        for _ in gN:
                            pass
    nc.compile()
    return nc


def _host_prep(inputs):
    """Precompute permuted/transposed weights and folded rope tables."""
    bf16 = np.float16
    perm = np.concatenate([np.arange(0, D, 2), np.arange(1, D, 2)])
    swap = np.concatenate([np.arange(D // 2, D), np.arange(0, D // 2)])

    qkv_w = np.asarray(inputs["qkv_w"], np.float32)
    rope = np.asarray(inputs["rope"], np.float32)
    sin_t, cos_t = rope[:, :D], rope[:, D:]

    # column order: [half][q|k|v][head-in-half][d]  (d permuted for q,k)
    row_order = np.empty(3 * C, np.int64)
    col = 0
    for hh in range(HH):
        for grp in range(3):
            for h in range(hh * HPH, (hh + 1) * HPH):
                base = grp * C + h * D
                idx = base + (perm if grp < 2 else np.arange(D))
                row_order[col:col + D] = idx
                col += D
    wt = np.ascontiguousarray(qkv_w[row_order, :].T).astype(bf16)  # [C, 3C]

    qb = np.asarray(inputs["q_bias"], np.float32)
    kb = np.asarray(inputs["k_bias"], np.float32)
    vb = np.asarray(inputs["v_bias"], np.float32)
    full_bias = np.concatenate([qb, kb, vb])
    qkvb = full_bias[row_order].astype(bf16)

    def make_tables(g, scale):
        gp = np.asarray(g, np.float32)[perm]
        cos_p = cos_t[:, perm]
        sin_p = sin_t[:, perm]
        sgn = np.where(np.arange(D) < D // 2, -1.0, 1.0).astype(np.float32)
        cost = np.zeros((NPAD, D), np.float32)
        sint = np.zeros((NPAD, D), np.float32)
        cost[0] = gp * scale
        cost[1:N] = cos_p * gp[None, :] * scale
        sint[1:N] = sin_p * sgn[None, :] * gp[swap][None, :] * scale
        return cost, sint

    cq, sq_ = make_tables(inputs["qn_g"], SCALE)
    ck, sk = make_tables(inputs["kn_g"], 1.0)
    ropet = np.stack([cq, sq_, ck, sk]).astype(bf16)  # [4, NPAD, D]

    norm_g = np.asarray(inputs["norm_g"], np.float32)
    norm_b = np.asarray(inputs["norm_b"], np.float32)
    proj_w = np.asarray(inputs["proj_w"], np.float32)
    proj_b = np.asarray(inputs["proj_b"], np.float32)
    pwt = np.ascontiguousarray((proj_w * norm_g[None, :]).T).astype(bf16)
    pbias = (proj_b + norm_b @ proj_w.T).astype(np.float32)

    return wt, qkvb, ropet, pwt, pbias


def kernel(**inputs):
    import ml_dtypes
    qn_b = np.asarray(inputs["qn_b"], np.float32)
    kn_b = np.asarray(inputs["kn_b"], np.float32)
    assert not qn_b.any() and not kn_b.any(), \
        "kernel specialized for qn_b == kn_b == 0"

    wt, qkvb, ropet, pwt, pbias = _host_prep(inputs)
    has_kbias = bool(np.asarray(inputs["k_bias"], np.float32).any())
    has_pbias = bool(pbias.any())

    key = (has_kbias, has_pbias)
    if key not in _CACHE:
        _CACHE[key] = _build(has_kbias, has_pbias)
    nc = _CACHE[key]

    x = np.asarray(inputs["x"], np.float32)
    in_maps = []
    for c in range(NCORES):
        in_maps.append({
            "x": np.ascontiguousarray(x[c * BL:(c + 1) * BL]),
            "wt": wt, "qkvb": qkvb, "ropet": ropet, "pwt": pwt,
            "pbias": pbias, "ident": np.eye(128, dtype=np.float32),
            "onesd": np.ones(1, dtype=ml_dtypes.bfloat16),
        })
    res = run_bass_kernel_spmd(nc, in_maps, core_ids=list(range(NCORES)))
    out = np.concatenate([res.results[c]["y"] for c in range(NCORES)], axis=0)
    return out.astype(np.float32)
